# revision 7
# baseline (speedup 1.0000x reference)
"""CloudCrop multi-scale kernel for 8 TRN2 NeuronCores.

Sharding: 2048 (b, m) seed-rows split 8 ways -> core r owns b=r//4,
m in [(r%4)*256, (r%4)*256+256). Each core computes its 256 seeds'
full pipeline; weights replicated; no collectives.

Per-core point index space is ROTATED so own seeds sit at columns
0..255 (SPMD graph is identical across cores); a "rev" priority
vector encodes the ORIGINAL index order so the cylinder query still
selects first-16-by-original-index.

Math restructuring vs reference:
 - BN folded into conv weights/biases (host).
 - conv1 split: feature part F1 = (a1*W1_feat) @ feats precomputed per
   scale on PE once per core, then 16 selected columns per seed are
   GATHERED (gpsimd ap_gather) instead of gathering 512-dim raw
   features and re-doing the big matmul per seed.
 - xyz part of conv1 via a K=9 matmul on P64[(e,d), col] =
   rot[m,d,e]/r_s * (xyz[idx]-xyz[m])_d, built from one batched
   4-scale gather of xyz + 2 DVE ops.
 - cylinder query: d2h = ly^2+lz^2 + ramp penalties(lx); score =
   (d2h < r^2) * rev; two rounds of top-8 (max/max_index/match_replace)
   give the first-16 valid original indices; pads get the first valid
   index. Since downstream is per-column conv + max-pool, only the SET
   of selected columns matters.
 - maxpool moved BEFORE bias+relu of conv2 (valid since relu monotone,
   bias per-channel).
"""
import numpy as np
import ml_dtypes
import concourse.bacc as bacc
import concourse.mybir as mybir
from concourse.tile import TileContext
from concourse.bass_utils import run_bass_kernel_spmd

P = 128
B, M, N, CSEED = 2, 1024, 1024, 512
NS = 16
RADII = (0.0125, 0.025, 0.0375, 0.05)
HMIN, HMAX = -0.02, 0.04
EPS = 1e-5
SLOPE = 1e8  # height-gate ramp penalty slope
F32 = mybir.dt.float32
I16 = mybir.dt.int16
U16 = mybir.dt.uint16
U8 = mybir.dt.uint8
BF16 = mybir.dt.bfloat16
AL = mybir.AluOpType
AF = mybir.ActivationFunctionType

_CACHE = {}


def build_graph(reps=1):
    nc = bacc.Bacc()
    dp = nc.declare_dram_parameter

    feats = dp("feats", [CSEED, N], BF16, isOutput=False)
    feats_own = dp("feats_own", [CSEED, 256], BF16, isOutput=False)
    xyzT = dp("xyzT", [4, N], F32, isOutput=False)
    lloc = dp("lloc", [2, 3, 4, P], F32, isOutput=False)
    tab128 = dp("tab128", [P, N], F32, isOutput=False)
    sx128 = dp("sx128", [P, 256], F32, isOutput=False)
    rc128 = dp("rc128", [P, 256], F32, isOutput=False)
    w1x9 = dp("w1x9", [P, 256], BF16, isOutput=False)
    w1fT = dp("w1fT", [4, CSEED, 256], BF16, isOutput=False)
    b1v = dp("b1v", [4, 256, 1], F32, isOutput=False)
    w2T = dp("w2T", [4, 256, 256], BF16, isOutput=False)
    b2v = dp("b2v", [4, 256, 1], F32, isOutput=False)
    fuseT = dp("fuseT", [1024, 256], BF16, isOutput=False)
    fusebv = dp("fusebv", [256, 1], F32, isOutput=False)
    transT = dp("transT", [CSEED, 256], BF16, isOutput=False)
    transbv = dp("transbv", [256, 1], F32, isOutput=False)
    gateT = dp("gateT", [256, 256], BF16, isOutput=False)
    gatebv = dp("gatebv", [256, 1], F32, isOutput=False)
    ident = dp("ident", [P, P], F32, isOutput=False)
    identb = dp("identb", [P, P], BF16, isOutput=False)
    rep16 = dp("rep16", [16, P], F32, isOutput=False)
    iota1k = dp("iota1k", [1, N], I16, isOutput=False)
    rev = dp("rev", [1, N], I16, isOutput=False)
    iota16 = dp("iota16", [1, 16], F32, isOutput=False)
    out = dp("out", [256, 256], F32, isOutput=True)

    with TileContext(nc) as tc:
        with tc.tile_pool(name="const", bufs=1) as cn, \
             tc.tile_pool(name="f1", bufs=4) as f1p, \
             tc.tile_pool(name="sel", bufs=2) as selp, \
             tc.tile_pool(name="tiny", bufs=3) as tn, \
             tc.tile_pool(name="gx", bufs=1) as gxp, \
             tc.tile_pool(name="gath", bufs=6) as gp, \
             tc.tile_pool(name="idxp", bufs=8) as ixp, \
             tc.tile_pool(name="y1p", bufs=4) as y1pool, \
             tc.tile_pool(name="cat", bufs=18) as catp, \
             tc.tile_pool(name="chunk", bufs=1) as chp, \
             tc.tile_pool(name="psE", bufs=1, space="PSUM") as psE, \
             tc.tile_pool(name="psU", bufs=5, space="PSUM") as psU, \
             tc.tile_pool(name="psS", bufs=1, space="PSUM") as psS:

            for _rep in range(reps):
                # ---------------- constants to SBUF (geometry first: critical path) ----------------
                xyzT_sb = cn.tile([4, N], F32)
                nc.sync.dma_start(xyzT_sb, xyzT[:])
                lloc_sb = [[cn.tile([4, P], F32, tag=f"lloc{t_}{e}", name=f"lloc{t_}{e}") for e in range(3)]
                           for t_ in range(2)]
                for t_ in range(2):
                    for e in range(3):
                        nc.sync.dma_start(lloc_sb[t_][e], lloc[t_, e])
                tab128_sb = cn.tile([P, N], F32)
                nc.sync.dma_start(tab128_sb, tab128[:])
                sx128_sb = cn.tile([P, 256], F32)
                nc.sync.dma_start(sx128_sb, sx128[:])
                rc128_sb = cn.tile([P, 256], F32)
                nc.sync.dma_start(rc128_sb, rc128[:])
                iota1k_sb = cn.tile([P, N], I16)
                nc.sync.dma_start(iota1k_sb, iota1k[:].to_broadcast([P, N]))
                iota16_sb = cn.tile([P, 16], F32)
                nc.sync.dma_start(iota16_sb, iota16[:].to_broadcast([P, 16]))
                ident_sb = cn.tile([P, P], F32)
                nc.sync.dma_start(ident_sb, ident[:])
                identb_sb = cn.tile([P, P], BF16)
                nc.sync.dma_start(identb_sb, identb[:])
                rep16_sb = cn.tile([16, P], F32)
                nc.sync.dma_start(rep16_sb, rep16[:])
                zbias = cn.tile([P, 1], F32)
                nc.vector.memset(zbias, 0.0)
                pb1 = cn.tile([P, 1], F32)
                nc.vector.memset(pb1, -SLOPE * HMAX)
                pb2 = cn.tile([P, 1], F32)
                nc.vector.memset(pb2, SLOPE * HMIN)
                w1x9_sb = cn.tile([P, 256], BF16)
                nc.sync.dma_start(w1x9_sb, w1x9[:])
                w1fT_sb = [[cn.tile([P, 256], BF16, tag=f"w1fT{s}{kc}", name=f"w1fT{s}{kc}") for kc in range(4)]
                           for s in range(4)]
                for s in range(4):
                    for kc in range(4):
                        nc.sync.dma_start(w1fT_sb[s][kc], w1fT[s, kc * P:(kc + 1) * P, :])
                feats_sb = []
                for kc in range(4):
                    t = cn.tile([P, N], BF16, tag=f"feats{kc}", name=f"feats{kc}")
                    nc.sync.dma_start(t, feats[kc * P:(kc + 1) * P, :])
                    feats_sb.append(t)
                fo_sb = []
                for kc in range(4):
                    t = cn.tile([P, 256], BF16, tag=f"fo{kc}", name=f"fo{kc}")
                    nc.sync.dma_start(t, feats_own[kc * P:(kc + 1) * P, :])
                    fo_sb.append(t)
                w2T_sb = [[cn.tile([P, 256], BF16, tag=f"w2T{s}{kc}", name=f"w2T{s}{kc}") for kc in range(2)]
                          for s in range(4)]
                for s in range(4):
                    for kc in range(2):
                        nc.sync.dma_start(w2T_sb[s][kc], w2T[s, kc * P:(kc + 1) * P, :])
                b1_sb = [[cn.tile([P, 1], F32, tag=f"b1{s}{ct}", name=f"b1{s}{ct}") for ct in range(2)]
                         for s in range(4)]
                b2_sb = [[cn.tile([P, 1], F32, tag=f"b2{s}{ct}", name=f"b2{s}{ct}") for ct in range(2)]
                         for s in range(4)]
                for s in range(4):
                    for ct in range(2):
                        nc.sync.dma_start(b1_sb[s][ct], b1v[s, ct * P:(ct + 1) * P, :])
                        nc.sync.dma_start(b2_sb[s][ct], b2v[s, ct * P:(ct + 1) * P, :])
                fuseT_sb = [cn.tile([P, 256], BF16, tag=f"fuseT{kc}", name=f"fuseT{kc}") for kc in range(8)]
                for kc in range(8):
                    nc.sync.dma_start(fuseT_sb[kc], fuseT[kc * P:(kc + 1) * P, :])
                transT_sb = [cn.tile([P, 256], BF16, tag=f"transT{kc}", name=f"transT{kc}") for kc in range(4)]
                for kc in range(4):
                    nc.sync.dma_start(transT_sb[kc], transT[kc * P:(kc + 1) * P, :])
                gateT_sb = [cn.tile([P, 256], BF16, tag=f"gateT{kc}", name=f"gateT{kc}") for kc in range(2)]
                for kc in range(2):
                    nc.sync.dma_start(gateT_sb[kc], gateT[kc * P:(kc + 1) * P, :])
                fuseb_sb = [cn.tile([P, 1], F32, tag=f"fb{ot}", name=f"fb{ot}") for ot in range(2)]
                transb_sb = [cn.tile([P, 1], F32, tag=f"tb{ot}", name=f"tb{ot}") for ot in range(2)]
                gateb_sb = [cn.tile([P, 1], F32, tag=f"gb{ot}", name=f"gb{ot}") for ot in range(2)]
                for ot in range(2):
                    sl = slice(ot * P, (ot + 1) * P)
                    nc.sync.dma_start(fuseb_sb[ot], fusebv[sl, :])
                    nc.sync.dma_start(transb_sb[ot], transbv[sl, :])
                    nc.sync.dma_start(gateb_sb[ot], gatebv[sl, :])

                # ------------- per-tile: local frames+d2h, selection, gathers -------------
                d2h = [cn.tile([P, N], F32, tag=f"d2h{t_}", name=f"d2h{t_}") for t_ in range(2)]
                idx128 = [tn.tile([P, P], I16, tag=f"idx128_{t_}", name=f"idx128_{t_}") for t_ in range(2)]
                idxr = {}
                gx64 = [None, None]
                for t_ in range(2):
                    for h in range(2):
                        hs = slice(h * 512, (h + 1) * 512)
                        sqy = chp.tile([P, 512], F32, tag="sqy")
                        sqz = chp.tile([P, 512], F32, tag="sqz")
                        pn1 = sqy
                        pn2 = sqz
                        dd = d2h[t_][:, hs]
                        for e in (1, 2, 0):
                            ps_loc = psE.tile([P, 512], F32, tag="early")
                            nc.tensor.matmul(ps_loc, lhsT=lloc_sb[t_][e],
                                             rhs=xyzT_sb[:, hs], start=True, stop=True)
                            if e == 1:
                                nc.scalar.activation(sqy, ps_loc, AF.Square, bias=zbias)
                            elif e == 2:
                                nc.scalar.activation(sqz, ps_loc, AF.Square, bias=zbias)
                                nc.vector.tensor_add(dd, sqy, sqz)
                            else:
                                nc.scalar.activation(pn1, ps_loc, AF.Relu,
                                                     scale=SLOPE, bias=pb1)
                                nc.vector.tensor_add(dd, dd, pn1)
                                nc.scalar.activation(pn2, ps_loc, AF.Relu,
                                                     scale=-SLOPE, bias=pb2)
                                nc.vector.tensor_add(dd, dd, pn2)
                    for s in range(4):
                        r2 = RADII[s] * RADII[s]
                        mask = selp.tile([P, N], I16, tag="mask")
                        nc.vector.tensor_scalar(out=mask, in0=d2h[t_], scalar1=r2,
                                                scalar2=None, op0=AL.is_lt)
                        rank = selp.tile([P, N], F32, tag="rank")
                        nc.vector.tensor_tensor_scan(out=rank, data0=mask, data1=mask,
                                                     initial=0.0, op0=AL.add, op1=AL.bypass)
                        nc.vector.tensor_mul(mask, rank, mask)
                        sidx = selp.tile([P, N], I16, tag="sidx")
                        nc.vector.tensor_scalar(out=sidx, in0=mask, scalar1=512.0,
                                                scalar2=1.0, op0=AL.min, op1=AL.subtract)
                        scat = selp.tile([P, 512], I16, tag="scat")
                        nc.gpsimd.local_scatter(out_ap=scat, data_ap=iota1k_sb, idxs_ap=sidx,
                                                channels=P, num_elems=512, num_idxs=N)
                        # pad invalid slots with first valid index
                        mif = tn.tile([P, 16], F32, tag="mif")
                        nc.vector.tensor_copy(mif, scat[:, 0:16])
                        validm = tn.tile([P, 16], U8, tag="validm")
                        nc.vector.tensor_scalar(out=validm, in0=iota16_sb,
                                                scalar1=rank[:, N - 1:N], scalar2=None,
                                                op0=AL.is_lt)
                        padded = tn.tile([P, 16], F32, tag="padded")
                        nc.vector.tensor_copy(padded, mif[:, 0:1].to_broadcast([P, 16]))
                        nc.vector.copy_predicated(out=padded, mask=validm, data=mif)
                        # wrapped [16,128] (= padded^T) and replicated [128,128] idx tiles
                        trps = psS.tile([16, P], F32, tag="tr", bufs=1)
                        nc.tensor.transpose(trps, padded, ident_sb)
                        trsb = tn.tile([16, P], F32, tag="trsb")
                        nc.vector.tensor_copy(trsb, trps)
                        repps = psS.tile([P, P], F32, tag="tr", bufs=1)
                        nc.tensor.matmul(repps, lhsT=rep16_sb, rhs=trsb, start=True, stop=True)
                        nc.vector.tensor_copy(idx128[t_][s * 32:(s + 1) * 32, :], repps[0:32, :])
                        ir = ixp.tile([P, P], I16, tag="idxr")
                        nc.vector.tensor_copy(ir, repps)
                        idxr[(s, t_)] = ir
                    # batched 4-scale xyz gather + P64 for this tile
                    # (2 calls of num_idxs=1024: 2048-idx gathers are ~4x
                    # slower per idx on HW)
                    g = gxp.tile([P, 2 * N], F32, tag=f"gx64_{t_}", name=f"gx64_{t_}")
                    nc.gpsimd.ap_gather(out_ap=g[:, 0:N], in_ap=tab128_sb,
                                        idxs_ap=idx128[t_][:, 0:64],
                                        channels=P, num_elems=N, d=1, num_idxs=1024)
                    nc.gpsimd.ap_gather(out_ap=g[:, N:2 * N], in_ap=tab128_sb,
                                        idxs_ap=idx128[t_][:, 64:128],
                                        channels=P, num_elems=N, d=1, num_idxs=1024)
                    gv = g.rearrange("p (m k) -> p m k", k=16)
                    sxv = sx128_sb[:, t_ * P:(t_ + 1) * P][:, :, None].to_broadcast([P, P, 16])
                    rcv = rc128_sb[:, t_ * P:(t_ + 1) * P][:, :, None].to_broadcast([P, P, 16])
                    nc.vector.tensor_sub(gv, gv, sxv)
                    pb = gxp.tile([P, 2 * N], BF16, tag=f"p64_{t_}", name=f"p64_{t_}")
                    nc.gpsimd.tensor_mul(pb.rearrange("p (m k) -> p m k", k=16), gv, rcv)
                    gx64[t_] = pb

                # ------------- phase A: F1fold per scale (PE) -------------
                f1sb = []
                for s in range(4):
                    t = f1p.tile([P, 2 * N], F32, tag="f1sb", name=f"f1sb{s}")
                    for ct in range(2):
                        for h in range(2):
                            hs = slice(h * 512, (h + 1) * 512)
                            ps_f1 = psU.tile([P, 512], F32, tag="unit")
                            for kc in range(4):
                                nc.tensor.matmul(
                                    ps_f1, lhsT=w1fT_sb[s][kc][:, ct * P:(ct + 1) * P],
                                    rhs=feats_sb[kc][:, hs],
                                    start=(kc == 0), stop=(kc == 3))
                            nc.scalar.activation(t[:, ct * N + h * 512: ct * N + (h + 1) * 512],
                                                 ps_f1, AF.Copy)
                    f1sb.append(t)

                # ------------- per (t, s): gather F1, conv1, conv2, pool -------------
                catk = {}
                catkb = {}
                for t_ in range(2):
                    for s in range(4):
                        f1g = {}
                        for ct in range(2):
                            for h2 in range(2):
                                g = gp.tile([P, 1024], F32, tag="f1g", name=f"f1g{ct}{h2}", bufs=6)
                                nc.gpsimd.ap_gather(out_ap=g, in_ap=f1sb[s][:, ct * N:(ct + 1) * N],
                                                    idxs_ap=idxr[(s, t_)][:, h2 * 64:(h2 + 1) * 64],
                                                    channels=P, num_elems=N, d=1, num_idxs=1024)
                                f1g[(ct, h2)] = g
                        for ch in range(4):
                            hs = slice((ch % 2) * 512, (ch % 2) * 512 + 512)
                            gs = slice(ch * 512, (ch + 1) * 512)
                            y1 = []
                            for ct in range(2):
                                ps_y1 = psU.tile([P, 512], F32, tag="unit")
                                nc.tensor.matmul(ps_y1, lhsT=ident_sb, rhs=f1g[(ct, ch // 2)][:, hs],
                                                 start=True, stop=False)
                                nc.tensor.matmul(
                                    ps_y1,
                                    lhsT=w1x9_sb[s * 32:s * 32 + 9, ct * P:(ct + 1) * P],
                                    rhs=gx64[t_][s * 32:s * 32 + 9, gs],
                                    start=False, stop=True, tile_position=(s * 32, 0))
                                yt = y1pool.tile([P, 512], BF16, tag="y1")
                                nc.scalar.activation(yt, ps_y1, AF.Relu, bias=b1_sb[s][ct])
                                y1.append(yt)
                            for ot in range(2):
                                key = (s, t_, ot)
                                if key not in catk:
                                    catk[key] = catp.tile([P, P], F32, tag="catk", name=f"catk{s}{t_}{ot}")
                                ps_y2 = psU.tile([P, 512], F32, tag="unit")
                                for kc in range(2):
                                    nc.tensor.matmul(ps_y2,
                                                     lhsT=w2T_sb[s][kc][:, ot * P:(ot + 1) * P],
                                                     rhs=y1[kc], start=(kc == 0), stop=(kc == 1))
                                nc.vector.tensor_reduce(
                                    out=catk[key][:, ch * 32:(ch + 1) * 32],
                                    in_=ps_y2.rearrange("p (g k) -> p g k", k=16),
                                    axis=mybir.AxisListType.X, op=AL.max)
                        for ot in range(2):
                            cb = catp.tile([P, P], BF16, tag="catkb", name=f"catkb{s}{t_}{ot}")
                            nc.scalar.activation(cb, catk[(s, t_, ot)],
                                                 AF.Relu, bias=b2_sb[s][ot])
                            catkb[(s, t_, ot)] = cb

                    # ---- fuse + gate + output for this m-tile ----
                    st = []
                    st_b = []
                    for ot in range(2):
                        ps_st = psS.tile([P, P], F32, tag="small")
                        for kc in range(4):
                            nc.tensor.matmul(ps_st, lhsT=transT_sb[kc][:, ot * P:(ot + 1) * P],
                                             rhs=fo_sb[kc][:, t_ * P:(t_ + 1) * P],
                                             start=(kc == 0), stop=(kc == 3))
                        stt_ = tn.tile([P, P], F32, tag="st")
                        nc.vector.tensor_scalar(out=stt_, in0=ps_st, scalar1=transb_sb[ot],
                                                scalar2=None, op0=AL.add)
                        stb = tn.tile([P, P], BF16, tag="stb", name=f"stb{ot}")
                        nc.vector.tensor_copy(stb, stt_)
                        st.append(stt_)
                        st_b.append(stb)
                    for ot in range(2):
                        ps_g = psS.tile([P, P], F32, tag="small")
                        for kc in range(2):
                            nc.tensor.matmul(ps_g, lhsT=gateT_sb[kc][:, ot * P:(ot + 1) * P],
                                             rhs=st_b[kc], start=(kc == 0), stop=(kc == 1))
                        gsig = tn.tile([P, P], F32, tag="gsig")
                        nc.scalar.activation(gsig, ps_g, AF.Sigmoid, bias=gateb_sb[ot])
                        ps_fu = psS.tile([P, P], F32, tag="small")
                        for kc in range(8):
                            s_, ot2 = divmod(kc, 2)
                            nc.tensor.matmul(ps_fu, lhsT=fuseT_sb[kc][:, ot * P:(ot + 1) * P],
                                             rhs=catkb[(s_, t_, ot2)],
                                             start=(kc == 0), stop=(kc == 7))
                        t1 = tn.tile([P, P], F32, tag="t1")
                        nc.vector.tensor_mul(t1, gsig, st[ot])
                        ob = tn.tile([P, P], F32, tag="ob")
                        nc.vector.scalar_tensor_tensor(out=ob, in0=t1, scalar=fuseb_sb[ot],
                                                       in1=ps_fu, op0=AL.add, op1=AL.add)
                        nc.sync.dma_start(out[ot * P:(ot + 1) * P, t_ * P:(t_ + 1) * P], ob)



    nc.compile()
    return nc


def _host_prep(inputs):
    """Fold BN, transpose weights, build per-core arrays."""
    f32 = np.float32
    xyz = np.asarray(inputs["seed_xyz"], f32)
    feats = np.asarray(inputs["seed_features"], f32)
    rot = np.asarray(inputs["vp_rot"], f32)
    W1 = np.asarray(inputs["crop_W1"], f32)
    b1 = np.asarray(inputs["crop_b1"], f32)
    g1 = np.asarray(inputs["crop_g1"], f32)
    be1 = np.asarray(inputs["crop_be1"], f32)
    m1 = np.asarray(inputs["crop_m1"], f32)
    v1 = np.asarray(inputs["crop_v1"], f32)
    W2 = np.asarray(inputs["crop_W2"], f32)
    b2 = np.asarray(inputs["crop_b2"], f32)
    g2 = np.asarray(inputs["crop_g2"], f32)
    be2 = np.asarray(inputs["crop_be2"], f32)
    m2 = np.asarray(inputs["crop_m2"], f32)
    v2 = np.asarray(inputs["crop_v2"], f32)

    a1 = (g1 / np.sqrt(v1 + EPS)).astype(f32)          # (4,256)
    a2 = (g2 / np.sqrt(v2 + EPS)).astype(f32)
    b1tot = (a1 * (b1 - m1) + be1).astype(f32)
    b2tot = (a2 * (b2 - m2) + be2).astype(f32)

    w1x9 = np.zeros((P, 256), f32)
    w1fT = np.zeros((4, CSEED, 256), f32)
    w2T = np.zeros((4, 256, 256), f32)
    for s in range(4):
        W1x = a1[s][:, None] * W1[s][:, 0:3]            # (256,3)
        for e in range(3):
            for d in range(3):
                w1x9[s * 32 + e * 3 + d] = W1x[:, e]
        w1fT[s] = (a1[s][:, None] * W1[s][:, 3:]).T
        w2T[s] = (a2[s][:, None] * W2[s]).T

    fuseT = np.asarray(inputs["fuse_W"], f32).T.copy()
    transT = np.asarray(inputs["trans_W"], f32).T.copy()
    gateT = np.asarray(inputs["gate_W"], f32).T.copy()
    fuseb = np.asarray(inputs["fuse_b"], f32).reshape(256, 1)
    transb = np.asarray(inputs["trans_b"], f32).reshape(256, 1)
    gateb = np.asarray(inputs["gate_b"], f32).reshape(256, 1)

    ident = np.eye(P, dtype=f32)
    rep16 = np.zeros((16, P), f32)
    for p in range(P):
        rep16[p % 16, p] = 1.0

    in_maps = []
    for r in range(8):
        b, q = divmod(r, 4)
        coff = q * 256
        xb = xyz[b]                                     # (1024,3)
        xo = xyz[b][coff:coff + 256]                    # (256,3) own
        ro = rot[b][coff:coff + 256]                    # (256,3,3) own

        xyzT = np.concatenate([xb.T, np.ones((1, N), f32)], 0)
        lloc = np.zeros((2, 3, 4, P), f32)
        for t_ in range(2):
            sl = slice(t_ * P, (t_ + 1) * P)
            for e in range(3):
                lloc[t_, e, 0:3, :] = ro[sl, :, e].T
                lloc[t_, e, 3, :] = -(xo[sl] * ro[sl, :, e]).sum(-1)
        tab128 = np.zeros((P, N), f32)
        sx128 = np.zeros((P, 256), f32)
        rc128 = np.zeros((P, 256), f32)
        for s in range(4):
            for e in range(3):
                for d in range(3):
                    rr = e * 3 + d
                    tab128[s * 32 + rr] = xb[:, d]
                    sx128[s * 32 + rr] = xo[:, d]
                    rc128[s * 32 + rr] = ro[:, d, e] / RADII[s]
        bf = ml_dtypes.bfloat16
        in_maps.append(dict(
            feats=feats[b].astype(bf),
            feats_own=feats[b][:, coff:coff + 256].astype(bf),
            xyzT=xyzT, lloc=lloc,
            tab128=tab128, sx128=sx128, rc128=rc128, w1x9=w1x9.astype(bf),
            w1fT=w1fT.astype(bf),
            b1v=b1tot.reshape(4, 256, 1), w2T=w2T.astype(bf),
            b2v=b2tot.reshape(4, 256, 1),
            fuseT=fuseT.astype(bf), fusebv=fuseb, transT=transT.astype(bf),
            transbv=transb, gateT=gateT.astype(bf), gatebv=gateb,
            ident=ident, identb=ident.astype(bf), rep16=rep16,
            iota1k=np.arange(N, dtype=np.int16).reshape(1, N),
            rev=(N - np.arange(N, dtype=np.int16)).reshape(1, N),
            iota16=np.arange(16, dtype=f32).reshape(1, 16)))
    return in_maps


def kernel(**inputs) -> np.ndarray:
    if "nc" not in _CACHE:
        _CACHE["nc"] = build_graph()
    nc = _CACHE["nc"]
    in_maps = _host_prep(inputs)
    res = run_bass_kernel_spmd(nc, in_maps, list(range(8)))
    outf = np.zeros((B, 256, M), np.float32)
    for r in range(8):
        b, q = divmod(r, 4)
        outf[b, :, q * 256:(q + 1) * 256] = res.results[r]["out"]
    return outf

